# revision 26
# baseline (speedup 1.0000x reference)
"""Trainium2 Bass kernel for nn_Linear_48335561949661.

y = x @ dequant(weight, scale)^T
  x:      [4, 8, 7168] fp32
  weight: [18432, 7168] fp32 (block-dequantized by scale over 128x128 blocks)
  scale:  [144, 56] fp32
  y:      [4, 8, 18432] fp32

Sharding: column-parallel linear — weight/scale sharded along out_features
across 8 cores, x replicated, outputs concatenated on host.

Host packing applies the block-scale dequant and lays the weight shard out
as w^T tiles [i-in-block(128 part), o(free)] in the kernel's compute dtype,
so the device is a pure stream: SWDGE DMA of 56 K-tiles overlapped with 56
PSUM-accumulating matmuls (stationary x^T tile [128, 32], moving w^T strip),
then one eviction + output DMA. No on-device transposes or elementwise
dequant (a step-0 scale AP forces the DVE to 1x — measured 3.9us per
[128,3584] tile on the old pipeline — so the scale multiply stays on host).

fp8 variant: weights stored as e3m4 with a per-(core, K-tile) power-of-two
renorm absorbed into that core's x^T tile to keep blocks with small scale
out of the e3m4 subnormal floor. Moving operand feeds the PE directly at
1 col/cycle; halves the HBM stream vs fp16.
"""

import sys

sys.path.insert(0, "/opt/trn_rl_repo")

import numpy as np

import concourse.bass as bass
import concourse.tile as tile
from concourse import bacc, mybir

FP32 = mybir.dt.float32
FP16 = mybir.dt.float16
FP8E3 = mybir.dt.float8e3

BLOCK = 128  # dequant block size

# Full-problem constants (hardcoded per contract; kernel.py reads no files)
B, S, I, O = 4, 8, 7168, 18432
NCORES = 8
T = B * S                # 32 tokens
OSH = O // NCORES        # 2304 out rows per core
NIB = I // BLOCK         # 56 K-tiles

# compute dtype for the weight stream: "fp16", "fp8e3", or "mixed"
WDT = "mixed"
K16 = 12        # mixed: K-tiles (of 56) streamed in fp16, rest fp8e3
WQUEUE = "hw3"  # w queues: "gpsimd" | "hw2" (sync+act) | "hw3" (+gpsimd)


def _tile_dtypes(wdt, n_ib=NIB, k16=K16):
    """Per-K-tile dtype list. Mixed spreads the fp16 slots evenly; the host
    ranks tiles by fp8 quantization error and maps the worst into them."""
    if wdt == "fp16":
        return ["fp16"] * n_ib
    if wdt == "fp8e3":
        return ["fp8e3"] * n_ib
    slots = ["fp8e3"] * n_ib
    for j in range(k16):
        # spread evenly but keep slot 0 fp8 (smaller first tile -> the
        # first matmul starts sooner)
        slots[int((j + 1) * n_ib / (k16 + 1))] = "fp16"
    return slots


def build_nc(wdt=WDT, osh=OSH, t=T, n_ib=NIB, nw=12, o_split=512, k16=K16,
             wqueue=WQUEUE, pair8=True, debug=False):
    """Per-core Bass program (SPMD: same program, 8 data shards)."""
    tdts = _tile_dtypes(wdt, n_ib, k16)
    n16 = sum(1 for d in tdts if d == "fp16")
    n8 = n_ib - n16

    nc = bacc.Bacc("TRN2", target_bir_lowering=False, debug=debug)

    w16_d = (nc.dram_tensor("w16", [BLOCK, n16 * osh], FP16,
                            kind="ExternalInput") if n16 else None)
    w8_d = (nc.dram_tensor("w8", [BLOCK, n8 * osh], FP8E3,
                           kind="ExternalInput") if n8 else None)
    # xt packed on host: xt[p, ib*t + tok] = x[tok, ib*128 + p] (renormed,
    # K-tiles permuted to match the w16/w8 slot assignment)
    xt_d = nc.dram_tensor("xt", [BLOCK, n_ib * t], FP16, kind="ExternalInput")
    y_d = nc.dram_tensor("y", [t, osh], FP32, kind="ExternalOutput")

    groups = [(o0, min(o_split, osh - o0)) for o0 in range(0, osh, o_split)]

    with tile.TileContext(nc) as tc:
        with (
            tc.tile_pool(name="const", bufs=1) as const_pool,
            tc.tile_pool(name="psum", bufs=1, space="PSUM") as psum_pool,
        ):
            np8 = max(2, (nw + 1) // 2)  # fp8 ring: pair slots (2 tiles)
            xt_sb = const_pool.tile([BLOCK, n_ib * t], FP16, tag="xt")
            r16 = (const_pool.tile([BLOCK, osh * min(nw, n16)], FP16,
                                   tag="r16", name="r16") if n16 else None)
            r8 = (const_pool.tile(
                [BLOCK, osh * (2 * np8 if pair8 else min(nw, n8))], FP8E3,
                tag="r8", name="r8") if n8 else None)
            y_sb = const_pool.tile([t, osh], FP32, tag="ysb")
            # xt in two chunks so the first matmuls aren't gated on the
            # full 459KB load; each chunk's PE wait rides a warmup-absorber
            # ldweights so every matmul keeps a single sync wait (ISA limit)
            xt_split = 16 * t
            nc.sync.dma_start(xt_sb[:, 0:xt_split], xt_d.ap()[:, 0:xt_split])
            nc.sync.dma_start(xt_sb[:, xt_split:], xt_d.ap()[:, xt_split:])
            nc.tensor.ldweights(xt_sb[:, 0:t])

            pys = [psum_pool.tile([t, w], FP32, tag=f"py{g}", name=f"py{g}")
                   for g, (_, w) in enumerate(groups)]

            # fp8 bulk on SWDGE (saturates the DMA engine bus ~390GB/s);
            # fp16 tiles alternate across the two HWDGE queues (each caps

            # at ~145GB/s of descriptor dispatch, but their demand is low)
            if wqueue == "gpsimd":
                eng8, eng16s = nc.gpsimd, [nc.gpsimd]
            elif wqueue == "hw2":
                eng8, eng16s = nc.sync, [nc.scalar]
            else:
                eng8, eng16s = nc.gpsimd, [nc.sync, nc.scalar]
            wdma_names = set()

            def prune(dma):
                # drop DMA->DMA WAW vs the slot's previous fill: the reader
                # matmul's WAR edge (kept) already subsumes it
                for dep in list(dma.ins.sync_dependency_names()):
                    if dep in wdma_names:
                        dma.ins.try_remove_dependency(dep)
                wdma_names.add(dma.ins.name)

            half = (osh // 2 // o_split) * o_split
            i16 = i8 = 0
            for ib, tdt in enumerate(tdts):
                if tdt == "fp16":
                    rn = min(nw, n16)
                    wt = r16[:, (i16 % rn) * osh:(i16 % rn + 1) * osh]
                    src_ap = w16_d.ap()[:, i16 * osh:(i16 + 1) * osh]
                    # split across both HWDGE queues (at a matmul group
                    # boundary so each matmul keeps a single sync wait)
                    if len(eng16s) == 2:
                        prune(eng16s[i16 % 2].dma_start(
                            wt[:, 0:half], src_ap[:, 0:half]))
                        prune(eng16s[(i16 + 1) % 2].dma_start(
                            wt[:, half:], src_ap[:, half:]))
                    else:
                        prune(eng16s[0].dma_start(wt, src_ap))
                    i16 += 1
                elif pair8:
                    # fp8 stream in 2-tile DMAs: halves the per-DMA
                    # overhead (desc-gen, trigger, DGE delay, sem) so the
                    # SWDGE cadence stays ahead of the PE
                    pi, lane = divmod(i8, 2)
                    base = (pi % np8) * 2 * osh
                    wt = r8[:, base + lane * osh:base + (lane + 1) * osh]
                    src_ap = w8_d.ap()[:, i8 * osh:(i8 + 1) * osh]
                    if i8 < 4:
                        # pipeline fill: the first fp8 tiles go as halved
                        # single-tile DMAs, spread over SWDGE and the
                        # second HWDGE queue, so the PE can start early
                        # without outrunning the delivery schedule
                        feng = (eng8 if i8 % 2 == 0 or len(eng16s) < 2
                                else eng16s[1])
                        prune(feng.dma_start(
                            wt[:, 0:half], src_ap[:, 0:half]))
                        prune(feng.dma_start(wt[:, half:], src_ap[:, half:]))
                    elif lane == 0:
                        n_in_pair = min(2, n8 - i8)
                        prune(eng8.dma_start(
                            r8[:, base:base + n_in_pair * osh],
                            w8_d.ap()[:, i8 * osh:(i8 + n_in_pair) * osh]))
                    i8 += 1
                else:
                    rn = min(nw, n8)
                    wt = r8[:, (i8 % rn) * osh:(i8 % rn + 1) * osh]
                    src_ap = w8_d.ap()[:, i8 * osh:(i8 + 1) * osh]
                    prune(eng8.dma_start(wt, src_ap))
                    i8 += 1
                if ib * t == xt_split:
                    # absorber for the second xt chunk's DMA wait
                    nc.tensor.ldweights(xt_sb[:, xt_split:xt_split + t])
                for g, (o0, w) in enumerate(groups):
                    nc.tensor.matmul(
                        pys[g][:, :],
                        xt_sb[:, ib * t:(ib + 1) * t],
                        wt[:, o0:o0 + w],
                        start=(ib == 0),
                        stop=(ib == n_ib - 1),
                    )
            # even groups (incl. the last) evict on ACT and ship on the act
            # HWDGE queue — same-engine chains, no cross-engine sem on the
            # critical tail
            y_engs = ([nc.scalar, nc.sync] if wqueue == "hw3"
                      else [nc.sync, nc.sync])
            for g, (o0, w) in enumerate(groups):
                yo = y_sb[:, o0:o0 + w]
                if g % 2 == 0:
                    nc.scalar.activation(
                        yo, pys[g][:], mybir.ActivationFunctionType.Copy)
                else:
                    nc.vector.tensor_copy(yo, pys[g][:])
                y_engs[g % 2].dma_start(y_d.ap()[:, o0:o0 + w], yo)

    nc.compile()
    return nc


def _legalize_waits(nc):
    """TRN2 ISA structs encode a single sync wait. Drop waits implied by
    queue FIFO: SWDGE same-queue DMA writes are ordered by the descriptor
    ring, so a w-load DMA's DMASW lane wait is redundant once its
    cross-engine WAR wait is kept."""
    import bass_rust

    seq_ok = {"InstDrain", "InstEventSemaphore", "InstNoOp", "InstISA",
              "InstCall", "InstUnconditionalBranch", "InstRegisterMove"}
    for fn in nc.m.functions:
        for bb in fn.blocks:
            for ins in bb.instructions:
                nm = type(ins).__name__
                si = ins.sync_info
                if not si or len(si.on_wait) <= 1 or nm in seq_ok:
                    continue
                waits = list(si.on_wait)
                if nm == "InstDMACopy":
                    keep = [w for w in waits
                            if not w.ant_name.startswith("DMASW")]
                    if len(keep) <= 1:
                        ins.sync_info = bass_rust.SyncInfo(
                            on_wait=keep, on_update=list(si.on_update))
                        continue
                raise RuntimeError(
                    f"unlegalizable multi-wait {nm} {ins.name}: "
                    f"{[w.ant_name for w in waits]}")


def _pack_inputs(x, weight, scale, wdt=WDT, osh=OSH, ncores=NCORES, k16=K16):
    """Host-side shard + dequant + repack. Returns per-core input maps.

    Mixed mode: per core, tiles are ranked by fp8e3 quantization error and
    the worst n16 stream as fp16; the program's K-tile order is a host-chosen
    permutation (contraction is order-independent), with xt permuted (and
    renormed) to match.
    """
    import ml_dtypes
    n_ib = NIB
    n_ob = osh // BLOCK
    t = T
    tdts = _tile_dtypes(wdt, n_ib, k16)
    slots16 = [i for i, d in enumerate(tdts) if d == "fp16"]
    slots8 = [i for i, d in enumerate(tdts) if d == "fp8e3"]
    n16 = len(slots16)
    xf = np.asarray(x, dtype=np.float32).reshape(t, I)
    # xt[p, ib*t+tok] = xf[tok, ib*128+p]
    xt_base = np.ascontiguousarray(
        xf.T.reshape(n_ib, BLOCK, t)
    )  # [n_ib, 128, t] fp32 (permute/renorm per core below)
    weight = np.asarray(weight, dtype=np.float32)
    scale = np.asarray(scale, dtype=np.float32)
    in_maps = []
    for c in range(ncores):
        wsh = weight[c * osh:(c + 1) * osh]            # [osh, I]
        ssh = scale[c * n_ob:(c + 1) * n_ob]           # [n_ob, n_ib]
        wd = (wsh.reshape(n_ob, BLOCK, n_ib, BLOCK)
              * ssh[:, None, :, None]).reshape(osh, I)
        # w^T tiles: wt[ib, p, o] = wd[o, ib*128 + p]
        wt = np.ascontiguousarray(wd.T.reshape(n_ib, BLOCK, osh))
        if n16 == n_ib:
            perm16, perm8 = list(range(n_ib)), []
        else:
            # per-(core, K-tile) power-of-two renorm keeps e3m4 blocks out
            # of the subnormal floor; compensated in this core's xt
            amax = np.maximum(np.abs(wt).max(axis=(1, 2)), 1e-30)
            f = np.exp2(np.floor(np.log2(13.0 / amax))).astype(np.float32)
            wq8 = ((wt * f[:, None, None]).astype(ml_dtypes.float8_e3m4)
                   .astype(np.float32) / f[:, None, None])
            err = ((wq8 - wt) ** 2).sum(axis=(1, 2))
            order = np.argsort(-err)
            perm16 = sorted(order[:n16].tolist())
            perm8 = sorted(order[n16:].tolist())
        xt_t = np.empty((n_ib, BLOCK, t), np.float32)
        m = {}
        if n16:
            w16 = np.ascontiguousarray(
                wt[perm16].transpose(1, 0, 2).reshape(BLOCK, n16 * osh)
            ).astype(np.float16)
            for j, ib in enumerate(perm16):
                xt_t[slots16[j]] = xt_base[ib]
            m["w16"] = w16
        if perm8:
            f8 = f[perm8]
            w8 = np.ascontiguousarray(
                (wt[perm8] * f8[:, None, None]).transpose(1, 0, 2)
                .reshape(BLOCK, len(perm8) * osh)
            ).astype(ml_dtypes.float8_e3m4)
            for j, ib in enumerate(perm8):
                xt_t[slots8[j]] = xt_base[ib] / f8[j]
            m["w8"] = w8
        m["xt"] = np.ascontiguousarray(
            xt_t.transpose(1, 0, 2).reshape(BLOCK, n_ib * t)
        ).astype(np.float16)
        in_maps.append(m)
    return in_maps


_NC_CACHE = {}


def _get_nc(**kw):
    key = tuple(sorted(kw.items()))
    if key not in _NC_CACHE:
        _NC_CACHE[key] = build_nc(**kw)
    return _NC_CACHE[key]


def _run(x, weight, scale, trace=False, wdt=WDT, k16=K16, nc_kw=None,
         **trace_kw):
    from concourse.bass_utils import run_bass_kernel_spmd

    nc = _get_nc(wdt=wdt, k16=k16, **(nc_kw or {}))
    in_maps = _pack_inputs(x, weight, scale, wdt=wdt, k16=k16)
    res = run_bass_kernel_spmd(
        nc, in_maps, core_ids=list(range(NCORES)), trace=trace, **trace_kw)
    y = np.concatenate([res.results[c]["y"] for c in range(NCORES)], axis=1)
    return np.ascontiguousarray(y.reshape(B, S, O).astype(np.float32)), res


def kernel(x, weight, scale):
    return _run(x, weight, scale)[0]


# revision 27
# speedup vs baseline: 1.0015x; 1.0015x over previous
"""Trainium2 Bass kernel for nn_Linear_48335561949661.

y = x @ dequant(weight, scale)^T
  x:      [4, 8, 7168] fp32
  weight: [18432, 7168] fp32 (block-dequantized by scale over 128x128 blocks)
  scale:  [144, 56] fp32
  y:      [4, 8, 18432] fp32

Sharding: column-parallel linear — weight/scale sharded along out_features
across 8 cores, x replicated, outputs concatenated on host.

Host packing applies the block-scale dequant and lays the weight shard out
as w^T tiles [i-in-block(128 part), o(free)] in the kernel's compute dtype,
so the device is a pure stream: SWDGE DMA of 56 K-tiles overlapped with 56
PSUM-accumulating matmuls (stationary x^T tile [128, 32], moving w^T strip),
then one eviction + output DMA. No on-device transposes or elementwise
dequant (a step-0 scale AP forces the DVE to 1x — measured 3.9us per
[128,3584] tile on the old pipeline — so the scale multiply stays on host).

fp8 variant: weights stored as e3m4 with a per-(core, K-tile) power-of-two
renorm absorbed into that core's x^T tile to keep blocks with small scale
out of the e3m4 subnormal floor. Moving operand feeds the PE directly at
1 col/cycle; halves the HBM stream vs fp16.
"""

import sys

sys.path.insert(0, "/opt/trn_rl_repo")

import numpy as np

import concourse.bass as bass
import concourse.tile as tile
from concourse import bacc, mybir

FP32 = mybir.dt.float32
FP16 = mybir.dt.float16
FP8E3 = mybir.dt.float8e3

BLOCK = 128  # dequant block size

# Full-problem constants (hardcoded per contract; kernel.py reads no files)
B, S, I, O = 4, 8, 7168, 18432
NCORES = 8
T = B * S                # 32 tokens
OSH = O // NCORES        # 2304 out rows per core
NIB = I // BLOCK         # 56 K-tiles

# compute dtype for the weight stream: "fp16", "fp8e3", or "mixed"
WDT = "mixed"
K16 = 12        # mixed: K-tiles (of 56) streamed in fp16, rest fp8e3
WQUEUE = "hw3"  # w queues: "gpsimd" | "hw2" (sync+act) | "hw3" (+gpsimd)


def _tile_dtypes(wdt, n_ib=NIB, k16=K16):
    """Per-K-tile dtype list. Mixed spreads the fp16 slots evenly; the host
    ranks tiles by fp8 quantization error and maps the worst into them."""
    if wdt == "fp16":
        return ["fp16"] * n_ib
    if wdt == "fp8e3":
        return ["fp8e3"] * n_ib
    slots = ["fp8e3"] * n_ib
    for j in range(k16):
        # spread evenly but keep slot 0 fp8 (smaller first tile -> the
        # first matmul starts sooner)
        slots[int((j + 1) * n_ib / (k16 + 1))] = "fp16"
    return slots


def build_nc(wdt=WDT, osh=OSH, t=T, n_ib=NIB, nw=8, o_split=512, k16=K16,
             wqueue=WQUEUE, pair8=True, debug=False):
    """Per-core Bass program (SPMD: same program, 8 data shards)."""
    tdts = _tile_dtypes(wdt, n_ib, k16)
    n16 = sum(1 for d in tdts if d == "fp16")
    n8 = n_ib - n16

    nc = bacc.Bacc("TRN2", target_bir_lowering=False, debug=debug)

    w16_d = (nc.dram_tensor("w16", [BLOCK, n16 * osh], FP16,
                            kind="ExternalInput") if n16 else None)
    w8_d = (nc.dram_tensor("w8", [BLOCK, n8 * osh], FP8E3,
                           kind="ExternalInput") if n8 else None)
    # xt packed on host: xt[p, ib*t + tok] = x[tok, ib*128 + p] (renormed,
    # K-tiles permuted to match the w16/w8 slot assignment)
    xt_d = nc.dram_tensor("xt", [BLOCK, n_ib * t], FP16, kind="ExternalInput")
    y_d = nc.dram_tensor("y", [t, osh], FP32, kind="ExternalOutput")

    groups = [(o0, min(o_split, osh - o0)) for o0 in range(0, osh, o_split)]

    with tile.TileContext(nc) as tc:
        with (
            tc.tile_pool(name="const", bufs=1) as const_pool,
            tc.tile_pool(name="psum", bufs=1, space="PSUM") as psum_pool,
        ):
            np8 = max(2, (nw + 1) // 2)  # fp8 ring: pair slots (2 tiles)
            xt_sb = const_pool.tile([BLOCK, n_ib * t], FP16, tag="xt")
            r16 = (const_pool.tile([BLOCK, osh * min(nw, n16)], FP16,
                                   tag="r16", name="r16") if n16 else None)
            r8 = (const_pool.tile(
                [BLOCK, osh * (2 * np8 if pair8 else min(nw, n8))], FP8E3,
                tag="r8", name="r8") if n8 else None)
            y_sb = const_pool.tile([t, osh], FP32, tag="ysb")
            # xt in two chunks so the first matmuls aren't gated on the
            # full 459KB load; each chunk's PE wait rides a warmup-absorber
            # ldweights so every matmul keeps a single sync wait (ISA limit)
            xt_split = 16 * t
            nc.sync.dma_start(xt_sb[:, 0:xt_split], xt_d.ap()[:, 0:xt_split])
            nc.sync.dma_start(xt_sb[:, xt_split:], xt_d.ap()[:, xt_split:])
            nc.tensor.ldweights(xt_sb[:, 0:t])

            pys = [psum_pool.tile([t, w], FP32, tag=f"py{g}", name=f"py{g}")
                   for g, (_, w) in enumerate(groups)]

            # fp8 bulk on SWDGE (saturates the DMA engine bus ~390GB/s);
            # fp16 tiles alternate across the two HWDGE queues (each caps

            # at ~145GB/s of descriptor dispatch, but their demand is low)
            if wqueue == "gpsimd":
                eng8, eng16s = nc.gpsimd, [nc.gpsimd]
            elif wqueue == "hw2":
                eng8, eng16s = nc.sync, [nc.scalar]
            else:
                eng8, eng16s = nc.gpsimd, [nc.sync, nc.scalar]
            wdma_names = set()

            def prune(dma):
                # drop DMA->DMA WAW vs the slot's previous fill: the reader
                # matmul's WAR edge (kept) already subsumes it
                for dep in list(dma.ins.sync_dependency_names()):
                    if dep in wdma_names:
                        dma.ins.try_remove_dependency(dep)
                wdma_names.add(dma.ins.name)

            half = (osh // 2 // o_split) * o_split
            i16 = i8 = 0
            for ib, tdt in enumerate(tdts):
                if tdt == "fp16":
                    rn = min(nw, n16)
                    wt = r16[:, (i16 % rn) * osh:(i16 % rn + 1) * osh]
                    src_ap = w16_d.ap()[:, i16 * osh:(i16 + 1) * osh]
                    # split across both HWDGE queues (at a matmul group
                    # boundary so each matmul keeps a single sync wait)
                    if len(eng16s) == 2:
                        prune(eng16s[i16 % 2].dma_start(
                            wt[:, 0:half], src_ap[:, 0:half]))
                        prune(eng16s[(i16 + 1) % 2].dma_start(
                            wt[:, half:], src_ap[:, half:]))
                    else:
                        prune(eng16s[0].dma_start(wt, src_ap))
                    i16 += 1
                elif pair8:
                    # fp8 stream in 2-tile DMAs: halves the per-DMA
                    # overhead (desc-gen, trigger, DGE delay, sem) so the
                    # SWDGE cadence stays ahead of the PE
                    pi, lane = divmod(i8, 2)
                    base = (pi % np8) * 2 * osh
                    wt = r8[:, base + lane * osh:base + (lane + 1) * osh]
                    src_ap = w8_d.ap()[:, i8 * osh:(i8 + 1) * osh]
                    if i8 < 4:
                        # pipeline fill: the first fp8 tiles go as halved
                        # single-tile DMAs, spread over SWDGE and the
                        # second HWDGE queue, so the PE can start early
                        # without outrunning the delivery schedule
                        feng = (eng8 if i8 % 2 == 0 or len(eng16s) < 2
                                else eng16s[1])
                        prune(feng.dma_start(
                            wt[:, 0:half], src_ap[:, 0:half]))
                        prune(feng.dma_start(wt[:, half:], src_ap[:, half:]))
                    elif lane == 0:
                        n_in_pair = min(2, n8 - i8)
                        prune(eng8.dma_start(
                            r8[:, base:base + n_in_pair * osh],
                            w8_d.ap()[:, i8 * osh:(i8 + n_in_pair) * osh]))
                    i8 += 1
                else:
                    rn = min(nw, n8)
                    wt = r8[:, (i8 % rn) * osh:(i8 % rn + 1) * osh]
                    src_ap = w8_d.ap()[:, i8 * osh:(i8 + 1) * osh]
                    prune(eng8.dma_start(wt, src_ap))
                    i8 += 1
                if ib * t == xt_split:
                    # absorber for the second xt chunk's DMA wait
                    nc.tensor.ldweights(xt_sb[:, xt_split:xt_split + t])
                for g, (o0, w) in enumerate(groups):
                    nc.tensor.matmul(
                        pys[g][:, :],
                        xt_sb[:, ib * t:(ib + 1) * t],
                        wt[:, o0:o0 + w],
                        start=(ib == 0),
                        stop=(ib == n_ib - 1),
                    )
            # even groups (incl. the last) evict on ACT and ship on the act
            # HWDGE queue — same-engine chains, no cross-engine sem on the
            # critical tail
            y_engs = ([nc.scalar, nc.sync] if wqueue == "hw3"
                      else [nc.sync, nc.sync])
            for g, (o0, w) in enumerate(groups):
                yo = y_sb[:, o0:o0 + w]
                if g % 2 == 0:
                    nc.scalar.activation(
                        yo, pys[g][:], mybir.ActivationFunctionType.Copy)
                else:
                    nc.vector.tensor_copy(yo, pys[g][:])
                y_engs[g % 2].dma_start(y_d.ap()[:, o0:o0 + w], yo)

    nc.compile()
    return nc


def _legalize_waits(nc):
    """TRN2 ISA structs encode a single sync wait. Drop waits implied by
    queue FIFO: SWDGE same-queue DMA writes are ordered by the descriptor
    ring, so a w-load DMA's DMASW lane wait is redundant once its
    cross-engine WAR wait is kept."""
    import bass_rust

    seq_ok = {"InstDrain", "InstEventSemaphore", "InstNoOp", "InstISA",
              "InstCall", "InstUnconditionalBranch", "InstRegisterMove"}
    for fn in nc.m.functions:
        for bb in fn.blocks:
            for ins in bb.instructions:
                nm = type(ins).__name__
                si = ins.sync_info
                if not si or len(si.on_wait) <= 1 or nm in seq_ok:
                    continue
                waits = list(si.on_wait)
                if nm == "InstDMACopy":
                    keep = [w for w in waits
                            if not w.ant_name.startswith("DMASW")]
                    if len(keep) <= 1:
                        ins.sync_info = bass_rust.SyncInfo(
                            on_wait=keep, on_update=list(si.on_update))
                        continue
                raise RuntimeError(
                    f"unlegalizable multi-wait {nm} {ins.name}: "
                    f"{[w.ant_name for w in waits]}")


def _pack_inputs(x, weight, scale, wdt=WDT, osh=OSH, ncores=NCORES, k16=K16):
    """Host-side shard + dequant + repack. Returns per-core input maps.

    Mixed mode: per core, tiles are ranked by fp8e3 quantization error and
    the worst n16 stream as fp16; the program's K-tile order is a host-chosen
    permutation (contraction is order-independent), with xt permuted (and
    renormed) to match.
    """
    import ml_dtypes
    n_ib = NIB
    n_ob = osh // BLOCK
    t = T
    tdts = _tile_dtypes(wdt, n_ib, k16)
    slots16 = [i for i, d in enumerate(tdts) if d == "fp16"]
    slots8 = [i for i, d in enumerate(tdts) if d == "fp8e3"]
    n16 = len(slots16)
    xf = np.asarray(x, dtype=np.float32).reshape(t, I)
    # xt[p, ib*t+tok] = xf[tok, ib*128+p]
    xt_base = np.ascontiguousarray(
        xf.T.reshape(n_ib, BLOCK, t)
    )  # [n_ib, 128, t] fp32 (permute/renorm per core below)
    weight = np.asarray(weight, dtype=np.float32)
    scale = np.asarray(scale, dtype=np.float32)
    in_maps = []
    for c in range(ncores):
        wsh = weight[c * osh:(c + 1) * osh]            # [osh, I]
        ssh = scale[c * n_ob:(c + 1) * n_ob]           # [n_ob, n_ib]
        wd = (wsh.reshape(n_ob, BLOCK, n_ib, BLOCK)
              * ssh[:, None, :, None]).reshape(osh, I)
        # w^T tiles: wt[ib, p, o] = wd[o, ib*128 + p]
        wt = np.ascontiguousarray(wd.T.reshape(n_ib, BLOCK, osh))
        if n16 == n_ib:
            perm16, perm8 = list(range(n_ib)), []
        else:
            # per-(core, K-tile) power-of-two renorm keeps e3m4 blocks out
            # of the subnormal floor; compensated in this core's xt
            amax = np.maximum(np.abs(wt).max(axis=(1, 2)), 1e-30)
            f = np.exp2(np.floor(np.log2(13.0 / amax))).astype(np.float32)
            wq8 = ((wt * f[:, None, None]).astype(ml_dtypes.float8_e3m4)
                   .astype(np.float32) / f[:, None, None])
            err = ((wq8 - wt) ** 2).sum(axis=(1, 2))
            order = np.argsort(-err)
            perm16 = sorted(order[:n16].tolist())
            perm8 = sorted(order[n16:].tolist())
        xt_t = np.empty((n_ib, BLOCK, t), np.float32)
        m = {}
        if n16:
            w16 = np.ascontiguousarray(
                wt[perm16].transpose(1, 0, 2).reshape(BLOCK, n16 * osh)
            ).astype(np.float16)
            for j, ib in enumerate(perm16):
                xt_t[slots16[j]] = xt_base[ib]
            m["w16"] = w16
        if perm8:
            f8 = f[perm8]
            w8 = np.ascontiguousarray(
                (wt[perm8] * f8[:, None, None]).transpose(1, 0, 2)
                .reshape(BLOCK, len(perm8) * osh)
            ).astype(ml_dtypes.float8_e3m4)
            for j, ib in enumerate(perm8):
                xt_t[slots8[j]] = xt_base[ib] / f8[j]
            m["w8"] = w8
        m["xt"] = np.ascontiguousarray(
            xt_t.transpose(1, 0, 2).reshape(BLOCK, n_ib * t)
        ).astype(np.float16)
        in_maps.append(m)
    return in_maps


_NC_CACHE = {}


def _get_nc(**kw):
    key = tuple(sorted(kw.items()))
    if key not in _NC_CACHE:
        _NC_CACHE[key] = build_nc(**kw)
    return _NC_CACHE[key]


def _run(x, weight, scale, trace=False, wdt=WDT, k16=K16, nc_kw=None,
         **trace_kw):
    from concourse.bass_utils import run_bass_kernel_spmd

    nc = _get_nc(wdt=wdt, k16=k16, **(nc_kw or {}))
    in_maps = _pack_inputs(x, weight, scale, wdt=wdt, k16=k16)
    res = run_bass_kernel_spmd(
        nc, in_maps, core_ids=list(range(NCORES)), trace=trace, **trace_kw)
    y = np.concatenate([res.results[c]["y"] for c in range(NCORES)], axis=1)
    return np.ascontiguousarray(y.reshape(B, S, O).astype(np.float32)), res


def kernel(x, weight, scale):
    return _run(x, weight, scale)[0]


# revision 28
# speedup vs baseline: 1.0266x; 1.0250x over previous
"""Trainium2 Bass kernel for nn_Linear_48335561949661.

y = x @ dequant(weight, scale)^T
  x:      [4, 8, 7168] fp32
  weight: [18432, 7168] fp32 (block-dequantized by scale over 128x128 blocks)
  scale:  [144, 56] fp32
  y:      [4, 8, 18432] fp32

Sharding: column-parallel linear — weight/scale sharded along out_features
across 8 cores, x replicated, outputs concatenated on host.

Host packing applies the block-scale dequant and lays the weight shard out
as w^T tiles [i-in-block(128 part), o(free)] in the kernel's compute dtype,
so the device is a pure stream: SWDGE DMA of 56 K-tiles overlapped with 56
PSUM-accumulating matmuls (stationary x^T tile [128, 32], moving w^T strip),
then one eviction + output DMA. No on-device transposes or elementwise
dequant (a step-0 scale AP forces the DVE to 1x — measured 3.9us per
[128,3584] tile on the old pipeline — so the scale multiply stays on host).

fp8 variant: weights stored as e3m4 with a per-(core, K-tile) power-of-two
renorm absorbed into that core's x^T tile to keep blocks with small scale
out of the e3m4 subnormal floor. Moving operand feeds the PE directly at
1 col/cycle; halves the HBM stream vs fp16.
"""

import sys

sys.path.insert(0, "/opt/trn_rl_repo")

import numpy as np

import concourse.bass as bass
import concourse.tile as tile
from concourse import bacc, mybir

FP32 = mybir.dt.float32
FP16 = mybir.dt.float16
FP8E3 = mybir.dt.float8e3

BLOCK = 128  # dequant block size

# Full-problem constants (hardcoded per contract; kernel.py reads no files)
B, S, I, O = 4, 8, 7168, 18432
NCORES = 8
T = B * S                # 32 tokens
OSH = O // NCORES        # 2304 out rows per core
NIB = I // BLOCK         # 56 K-tiles

# compute dtype for the weight stream: "fp16", "fp8e3", or "mixed"
WDT = "mixed"
K16 = 12        # mixed: K-tiles (of 56) streamed in fp16, rest fp8e3
WQUEUE = "hw3"  # w queues: "gpsimd" | "hw2" (sync+act) | "hw3" (+gpsimd)


def _tile_dtypes(wdt, n_ib=NIB, k16=K16):
    """Per-K-tile dtype list. Mixed spreads the fp16 slots evenly; the host
    ranks tiles by fp8 quantization error and maps the worst into them."""
    if wdt == "fp16":
        return ["fp16"] * n_ib
    if wdt == "fp8e3":
        return ["fp8e3"] * n_ib
    slots = ["fp8e3"] * n_ib
    for j in range(k16):
        # spread evenly but keep slot 0 fp8 (smaller first tile -> the
        # first matmul starts sooner)
        slots[int((j + 1) * n_ib / (k16 + 1))] = "fp16"
    return slots


def build_nc(wdt=WDT, osh=OSH, t=T, n_ib=NIB, nw=8, o_split=512, k16=K16,
             wqueue=WQUEUE, pair8=True, debug=False):
    """Per-core Bass program (SPMD: same program, 8 data shards)."""
    tdts = _tile_dtypes(wdt, n_ib, k16)
    n16 = sum(1 for d in tdts if d == "fp16")
    n8 = n_ib - n16

    nc = bacc.Bacc("TRN2", target_bir_lowering=False, debug=debug)

    w16_d = (nc.dram_tensor("w16", [BLOCK, n16 * osh], FP16,
                            kind="ExternalInput") if n16 else None)
    w8_d = (nc.dram_tensor("w8", [BLOCK, n8 * osh], FP8E3,
                           kind="ExternalInput") if n8 else None)
    # xt packed on host: xt[p, ib*t + tok] = x[tok, ib*128 + p] (renormed,
    # K-tiles permuted to match the w16/w8 slot assignment)
    xt_d = nc.dram_tensor("xt", [BLOCK, n_ib * t], FP16, kind="ExternalInput")
    y_d = nc.dram_tensor("y", [t, osh], FP32, kind="ExternalOutput")

    groups = [(o0, min(o_split, osh - o0)) for o0 in range(0, osh, o_split)]

    with tile.TileContext(nc) as tc:
        with (
            tc.tile_pool(name="const", bufs=1) as const_pool,
            tc.tile_pool(name="psum", bufs=1, space="PSUM") as psum_pool,
        ):
            np8 = max(2, (nw + 1) // 2)  # fp8 ring: pair slots (2 tiles)
            xt_sb = const_pool.tile([BLOCK, n_ib * t], FP16, tag="xt")
            r16 = (const_pool.tile([BLOCK, osh * min(nw, n16)], FP16,
                                   tag="r16", name="r16") if n16 else None)
            r8 = (const_pool.tile(
                [BLOCK, osh * (2 * np8 if pair8 else min(nw, n8))], FP8E3,
                tag="r8", name="r8") if n8 else None)
            y_sb = const_pool.tile([t, osh], FP32, tag="ysb")
            # xt in two chunks so the first matmuls aren't gated on the
            # full 459KB load; each chunk's PE wait rides a warmup-absorber
            # ldweights so every matmul keeps a single sync wait (ISA limit)
            xt_split = 16 * t
            nc.sync.dma_start(xt_sb[:, 0:xt_split], xt_d.ap()[:, 0:xt_split])
            nc.sync.dma_start(xt_sb[:, xt_split:], xt_d.ap()[:, xt_split:])
            nc.tensor.ldweights(xt_sb[:, 0:t])

            pys = [psum_pool.tile([t, w], FP32, tag=f"py{g}", name=f"py{g}")
                   for g, (_, w) in enumerate(groups)]

            # fp8 bulk on SWDGE (saturates the DMA engine bus ~390GB/s);
            # fp16 tiles alternate across the two HWDGE queues (each caps

            # at ~145GB/s of descriptor dispatch, but their demand is low)
            if wqueue == "gpsimd":
                eng8, eng16s = nc.gpsimd, [nc.gpsimd]
            elif wqueue == "hw2":
                eng8, eng16s = nc.sync, [nc.scalar]
            else:
                eng8, eng16s = nc.gpsimd, [nc.sync, nc.scalar]
            wdma_names = set()

            def prune(dma):
                # drop DMA->DMA WAW vs the slot's previous fill: the reader
                # matmul's WAR edge (kept) already subsumes it
                for dep in list(dma.ins.sync_dependency_names()):
                    if dep in wdma_names:
                        dma.ins.try_remove_dependency(dep)
                wdma_names.add(dma.ins.name)

            half = (osh // 2 // o_split) * o_split
            i16 = i8 = 0
            for ib, tdt in enumerate(tdts):
                if tdt == "fp16":
                    rn = min(nw, n16)
                    wt = r16[:, (i16 % rn) * osh:(i16 % rn + 1) * osh]
                    src_ap = w16_d.ap()[:, i16 * osh:(i16 + 1) * osh]
                    # halves split between SWDGE (not dispatch-limited) and
                    # the act HWDGE queue, at a matmul group boundary so
                    # each matmul keeps a single sync wait
                    if len(eng16s) == 2:
                        prune(eng8.dma_start(
                            wt[:, 0:half], src_ap[:, 0:half]))
                        prune(eng16s[1].dma_start(
                            wt[:, half:], src_ap[:, half:]))
                    else:
                        prune(eng16s[0].dma_start(wt, src_ap))
                    i16 += 1
                elif pair8:
                    # fp8 stream in 2-tile DMAs: halves the per-DMA
                    # overhead (desc-gen, trigger, DGE delay, sem) so the
                    # SWDGE cadence stays ahead of the PE
                    pi, lane = divmod(i8, 2)
                    base = (pi % np8) * 2 * osh
                    wt = r8[:, base + lane * osh:base + (lane + 1) * osh]
                    src_ap = w8_d.ap()[:, i8 * osh:(i8 + 1) * osh]
                    if i8 < 4:
                        # pipeline fill: the first fp8 tiles go as halved
                        # single-tile DMAs, spread over SWDGE and the
                        # second HWDGE queue, so the PE can start early
                        # without outrunning the delivery schedule
                        feng = (eng8 if i8 % 2 == 0 or len(eng16s) < 2
                                else eng16s[1])
                        prune(feng.dma_start(
                            wt[:, 0:half], src_ap[:, 0:half]))
                        prune(feng.dma_start(wt[:, half:], src_ap[:, half:]))
                    elif lane == 0:
                        n_in_pair = min(2, n8 - i8)
                        prune(eng8.dma_start(
                            r8[:, base:base + n_in_pair * osh],
                            w8_d.ap()[:, i8 * osh:(i8 + n_in_pair) * osh]))
                    i8 += 1
                else:
                    rn = min(nw, n8)
                    wt = r8[:, (i8 % rn) * osh:(i8 % rn + 1) * osh]
                    src_ap = w8_d.ap()[:, i8 * osh:(i8 + 1) * osh]
                    prune(eng8.dma_start(wt, src_ap))
                    i8 += 1
                if ib * t == xt_split:
                    # absorber for the second xt chunk's DMA wait
                    nc.tensor.ldweights(xt_sb[:, xt_split:xt_split + t])
                for g, (o0, w) in enumerate(groups):
                    nc.tensor.matmul(
                        pys[g][:, :],
                        xt_sb[:, ib * t:(ib + 1) * t],
                        wt[:, o0:o0 + w],
                        start=(ib == 0),
                        stop=(ib == n_ib - 1),
                    )
            # even groups (incl. the last) evict on ACT and ship on the act
            # HWDGE queue — same-engine chains, no cross-engine sem on the
            # critical tail
            y_engs = ([nc.scalar, nc.sync] if wqueue == "hw3"
                      else [nc.sync, nc.sync])
            for g, (o0, w) in enumerate(groups):
                yo = y_sb[:, o0:o0 + w]
                if g % 2 == 0:
                    nc.scalar.activation(
                        yo, pys[g][:], mybir.ActivationFunctionType.Copy)
                else:
                    nc.vector.tensor_copy(yo, pys[g][:])
                y_engs[g % 2].dma_start(y_d.ap()[:, o0:o0 + w], yo)

    nc.compile()
    return nc


def _legalize_waits(nc):
    """TRN2 ISA structs encode a single sync wait. Drop waits implied by
    queue FIFO: SWDGE same-queue DMA writes are ordered by the descriptor
    ring, so a w-load DMA's DMASW lane wait is redundant once its
    cross-engine WAR wait is kept."""
    import bass_rust

    seq_ok = {"InstDrain", "InstEventSemaphore", "InstNoOp", "InstISA",
              "InstCall", "InstUnconditionalBranch", "InstRegisterMove"}
    for fn in nc.m.functions:
        for bb in fn.blocks:
            for ins in bb.instructions:
                nm = type(ins).__name__
                si = ins.sync_info
                if not si or len(si.on_wait) <= 1 or nm in seq_ok:
                    continue
                waits = list(si.on_wait)
                if nm == "InstDMACopy":
                    keep = [w for w in waits
                            if not w.ant_name.startswith("DMASW")]
                    if len(keep) <= 1:
                        ins.sync_info = bass_rust.SyncInfo(
                            on_wait=keep, on_update=list(si.on_update))
                        continue
                raise RuntimeError(
                    f"unlegalizable multi-wait {nm} {ins.name}: "
                    f"{[w.ant_name for w in waits]}")


def _pack_inputs(x, weight, scale, wdt=WDT, osh=OSH, ncores=NCORES, k16=K16):
    """Host-side shard + dequant + repack. Returns per-core input maps.

    Mixed mode: per core, tiles are ranked by fp8e3 quantization error and
    the worst n16 stream as fp16; the program's K-tile order is a host-chosen
    permutation (contraction is order-independent), with xt permuted (and
    renormed) to match.
    """
    import ml_dtypes
    n_ib = NIB
    n_ob = osh // BLOCK
    t = T
    tdts = _tile_dtypes(wdt, n_ib, k16)
    slots16 = [i for i, d in enumerate(tdts) if d == "fp16"]
    slots8 = [i for i, d in enumerate(tdts) if d == "fp8e3"]
    n16 = len(slots16)
    xf = np.asarray(x, dtype=np.float32).reshape(t, I)
    # xt[p, ib*t+tok] = xf[tok, ib*128+p]
    xt_base = np.ascontiguousarray(
        xf.T.reshape(n_ib, BLOCK, t)
    )  # [n_ib, 128, t] fp32 (permute/renorm per core below)
    weight = np.asarray(weight, dtype=np.float32)
    scale = np.asarray(scale, dtype=np.float32)
    in_maps = []
    for c in range(ncores):
        wsh = weight[c * osh:(c + 1) * osh]            # [osh, I]
        ssh = scale[c * n_ob:(c + 1) * n_ob]           # [n_ob, n_ib]
        wd = (wsh.reshape(n_ob, BLOCK, n_ib, BLOCK)
              * ssh[:, None, :, None]).reshape(osh, I)
        # w^T tiles: wt[ib, p, o] = wd[o, ib*128 + p]
        wt = np.ascontiguousarray(wd.T.reshape(n_ib, BLOCK, osh))
        if n16 == n_ib:
            perm16, perm8 = list(range(n_ib)), []
        else:
            # per-(core, K-tile) power-of-two renorm keeps e3m4 blocks out
            # of the subnormal floor; compensated in this core's xt
            amax = np.maximum(np.abs(wt).max(axis=(1, 2)), 1e-30)
            f = np.exp2(np.floor(np.log2(13.0 / amax))).astype(np.float32)
            wq8 = ((wt * f[:, None, None]).astype(ml_dtypes.float8_e3m4)
                   .astype(np.float32) / f[:, None, None])
            err = ((wq8 - wt) ** 2).sum(axis=(1, 2))
            order = np.argsort(-err)
            perm16 = sorted(order[:n16].tolist())
            perm8 = sorted(order[n16:].tolist())
        xt_t = np.empty((n_ib, BLOCK, t), np.float32)
        m = {}
        if n16:
            w16 = np.ascontiguousarray(
                wt[perm16].transpose(1, 0, 2).reshape(BLOCK, n16 * osh)
            ).astype(np.float16)
            for j, ib in enumerate(perm16):
                xt_t[slots16[j]] = xt_base[ib]
            m["w16"] = w16
        if perm8:
            f8 = f[perm8]
            w8 = np.ascontiguousarray(
                (wt[perm8] * f8[:, None, None]).transpose(1, 0, 2)
                .reshape(BLOCK, len(perm8) * osh)
            ).astype(ml_dtypes.float8_e3m4)
            for j, ib in enumerate(perm8):
                xt_t[slots8[j]] = xt_base[ib] / f8[j]
            m["w8"] = w8
        m["xt"] = np.ascontiguousarray(
            xt_t.transpose(1, 0, 2).reshape(BLOCK, n_ib * t)
        ).astype(np.float16)
        in_maps.append(m)
    return in_maps


_NC_CACHE = {}


def _get_nc(**kw):
    key = tuple(sorted(kw.items()))
    if key not in _NC_CACHE:
        _NC_CACHE[key] = build_nc(**kw)
    return _NC_CACHE[key]


def _run(x, weight, scale, trace=False, wdt=WDT, k16=K16, nc_kw=None,
         **trace_kw):
    from concourse.bass_utils import run_bass_kernel_spmd

    nc = _get_nc(wdt=wdt, k16=k16, **(nc_kw or {}))
    in_maps = _pack_inputs(x, weight, scale, wdt=wdt, k16=k16)
    res = run_bass_kernel_spmd(
        nc, in_maps, core_ids=list(range(NCORES)), trace=trace, **trace_kw)
    y = np.concatenate([res.results[c]["y"] for c in range(NCORES)], axis=1)
    return np.ascontiguousarray(y.reshape(B, S, O).astype(np.float32)), res


def kernel(x, weight, scale):
    return _run(x, weight, scale)[0]


# revision 30
# speedup vs baseline: 1.0904x; 1.0622x over previous
"""Trainium2 Bass kernel for nn_Linear_48335561949661.

y = x @ dequant(weight, scale)^T
  x:      [4, 8, 7168] fp32
  weight: [18432, 7168] fp32 (block-dequantized by scale over 128x128 blocks)
  scale:  [144, 56] fp32
  y:      [4, 8, 18432] fp32

Sharding: column-parallel linear — weight/scale sharded along out_features
across 8 cores, x replicated, outputs concatenated on host.

Host packing applies the block-scale dequant and lays the weight shard out
as w^T K-tiles [i-in-block(128 part), o(free)], so the device is a pure
stream: DMA of 56 K-tiles overlapped with 56x5 PSUM-accumulating matmuls
(stationary x^T tile [128, 32], moving w^T strips of <=512 cols — the PSUM
bank cap), then a 5-chunk eviction + output DMA. No on-device transposes
or elementwise dequant (a step-0 scale AP forces the DVE to 1x, ~134us for
the full weight — so the scale multiply stays on host).

Weight stream dtype is mixed per K-tile: most tiles are fp8e3 (e3m4, fed
to the PE directly at 1 col/cycle) with a per-(core, K-tile) power-of-two
renorm absorbed into that core's x^T tile to dodge the e3m4 subnormal
floor; the K16 tiles with the worst fp8 quantization error stream as fp16
(host ranks per core, K-tile order is a host-chosen permutation). rel err
vs the fp32 reference: 1.1e-2 (k16=8), gate 2e-2 — numpy-simulated
bit-exact against hardware.

Queue layout (measured: one SWDGE queue sustains ~390GB/s; HWDGE queues
~145GB/s each, descriptor-dispatch-bound): fp8 pairs + fp16 first-halves
on SWDGE, fp16 second-halves on the act HWDGE queue, xt/y on sync. First
tiles go as halved single-tile DMAs spread over the queues so the PE
starts ~10.7us in; fp8 steady state ships as 2-tile DMAs to halve per-DMA
overhead. PE runs at its floor (~57.5us busy = 129k moving cols at
2.4GHz, LDWEIGHTS hidden); the stream is PE-bound with ~5us of residual
early-fill gaps. ~80us total vs the 250us session-start baseline.
"""

import sys

sys.path.insert(0, "/opt/trn_rl_repo")

import numpy as np

import concourse.bass as bass
import concourse.tile as tile
from concourse import bacc, mybir

FP32 = mybir.dt.float32
FP16 = mybir.dt.float16
FP8E3 = mybir.dt.float8e3

BLOCK = 128  # dequant block size

# Full-problem constants (hardcoded per contract; kernel.py reads no files)
B, S, I, O = 4, 8, 7168, 18432
NCORES = 8
T = B * S                # 32 tokens
OSH = O // NCORES        # 2304 out rows per core
NIB = I // BLOCK         # 56 K-tiles

# compute dtype for the weight stream: "fp16", "fp8e3", or "mixed"
WDT = "mixed"
K16 = 8         # mixed: K-tiles (of 56) streamed in fp16, rest fp8e3
WQUEUE = "hw3"  # w queues: "gpsimd" | "hw2" (sync+act) | "hw3" (+gpsimd)


def _tile_dtypes(wdt, n_ib=NIB, k16=K16):
    """Per-K-tile dtype list. Mixed spreads the fp16 slots evenly; the host
    ranks tiles by fp8 quantization error and maps the worst into them."""
    if wdt == "fp16":
        return ["fp16"] * n_ib
    if wdt == "fp8e3":
        return ["fp8e3"] * n_ib
    slots = ["fp8e3"] * n_ib
    for j in range(k16):
        # spread evenly but keep slot 0 fp8 (smaller first tile -> the
        # first matmul starts sooner)
        slots[int((j + 1) * n_ib / (k16 + 1))] = "fp16"
    return slots


def build_nc(wdt=WDT, osh=OSH, t=T, n_ib=NIB, nw=8, o_split=512, k16=K16,
             wqueue=WQUEUE, pair8=True, debug=False):
    """Per-core Bass program (SPMD: same program, 8 data shards)."""
    tdts = _tile_dtypes(wdt, n_ib, k16)
    n16 = sum(1 for d in tdts if d == "fp16")
    n8 = n_ib - n16

    nc = bacc.Bacc("TRN2", target_bir_lowering=False, debug=debug)

    w16_d = (nc.dram_tensor("w16", [BLOCK, n16 * osh], FP16,
                            kind="ExternalInput") if n16 else None)
    w8_d = (nc.dram_tensor("w8", [BLOCK, n8 * osh], FP8E3,
                           kind="ExternalInput") if n8 else None)
    # xt packed on host: xt[p, ib*t + tok] = x[tok, ib*128 + p] (renormed,
    # K-tiles permuted to match the w16/w8 slot assignment)
    xt_d = nc.dram_tensor("xt", [BLOCK, n_ib * t], FP16, kind="ExternalInput")
    y_d = nc.dram_tensor("y", [t, osh], FP32, kind="ExternalOutput")

    groups = [(o0, min(o_split, osh - o0)) for o0 in range(0, osh, o_split)]

    with tile.TileContext(nc) as tc:
        with (
            tc.tile_pool(name="const", bufs=1) as const_pool,
            tc.tile_pool(name="psum", bufs=1, space="PSUM") as psum_pool,
        ):
            np8 = max(2, (nw + 1) // 2)  # fp8 ring: pair slots (2 tiles)
            xt_sb = const_pool.tile([BLOCK, n_ib * t], FP16, tag="xt")
            r16 = (const_pool.tile([BLOCK, osh * min(nw, n16)], FP16,
                                   tag="r16", name="r16") if n16 else None)
            r8 = (const_pool.tile(
                [BLOCK, osh * (2 * np8 if pair8 else min(nw, n8))], FP8E3,
                tag="r8", name="r8") if n8 else None)
            y_sb = const_pool.tile([t, osh], FP32, tag="ysb")
            # xt in two chunks so the first matmuls aren't gated on the
            # full 459KB load; each chunk's PE wait rides a warmup-absorber
            # ldweights so every matmul keeps a single sync wait (ISA limit)
            xt_split = 16 * t
            nc.sync.dma_start(xt_sb[:, 0:xt_split], xt_d.ap()[:, 0:xt_split])
            nc.sync.dma_start(xt_sb[:, xt_split:], xt_d.ap()[:, xt_split:])
            nc.tensor.ldweights(xt_sb[:, 0:t])

            pys = [psum_pool.tile([t, w], FP32, tag=f"py{g}", name=f"py{g}")
                   for g, (_, w) in enumerate(groups)]

            # fp8 bulk on SWDGE (saturates the DMA engine bus ~390GB/s);
            # fp16 tiles alternate across the two HWDGE queues (each caps

            # at ~145GB/s of descriptor dispatch, but their demand is low)
            if wqueue == "gpsimd":
                eng8, eng16s = nc.gpsimd, [nc.gpsimd]
            elif wqueue == "hw2":
                eng8, eng16s = nc.sync, [nc.scalar]
            else:
                eng8, eng16s = nc.gpsimd, [nc.sync, nc.scalar]
            wdma_names = set()

            def prune(dma):
                # drop DMA->DMA WAW vs the slot's previous fill: the reader
                # matmul's WAR edge (kept) already subsumes it
                for dep in list(dma.ins.sync_dependency_names()):
                    if dep in wdma_names:
                        dma.ins.try_remove_dependency(dep)
                wdma_names.add(dma.ins.name)

            half = (osh // 2 // o_split) * o_split
            i16 = i8 = 0
            for ib, tdt in enumerate(tdts):
                if tdt == "fp16":
                    rn = min(nw, n16)
                    wt = r16[:, (i16 % rn) * osh:(i16 % rn + 1) * osh]
                    src_ap = w16_d.ap()[:, i16 * osh:(i16 + 1) * osh]
                    # halves split between SWDGE (not dispatch-limited) and
                    # the act HWDGE queue, at a matmul group boundary so
                    # each matmul keeps a single sync wait
                    if len(eng16s) == 2:
                        prune(eng8.dma_start(
                            wt[:, 0:half], src_ap[:, 0:half]))
                        prune(eng16s[1].dma_start(
                            wt[:, half:], src_ap[:, half:]))
                    else:
                        prune(eng16s[0].dma_start(wt, src_ap))
                    i16 += 1
                elif pair8:
                    # fp8 stream in 2-tile DMAs: halves the per-DMA
                    # overhead (desc-gen, trigger, DGE delay, sem) so the
                    # SWDGE cadence stays ahead of the PE
                    pi, lane = divmod(i8, 2)
                    base = (pi % np8) * 2 * osh
                    wt = r8[:, base + lane * osh:base + (lane + 1) * osh]
                    src_ap = w8_d.ap()[:, i8 * osh:(i8 + 1) * osh]
                    if i8 < 4:
                        # pipeline fill: the first fp8 tiles go as halved
                        # single-tile DMAs, spread over SWDGE and the
                        # second HWDGE queue, so the PE can start early
                        # without outrunning the delivery schedule
                        feng = (eng8 if i8 % 2 == 0 or len(eng16s) < 2
                                else eng16s[1])
                        prune(feng.dma_start(
                            wt[:, 0:half], src_ap[:, 0:half]))
                        prune(feng.dma_start(wt[:, half:], src_ap[:, half:]))
                    elif lane == 0:
                        n_in_pair = min(2, n8 - i8)
                        prune(eng8.dma_start(
                            r8[:, base:base + n_in_pair * osh],
                            w8_d.ap()[:, i8 * osh:(i8 + n_in_pair) * osh]))
                    i8 += 1
                else:
                    rn = min(nw, n8)
                    wt = r8[:, (i8 % rn) * osh:(i8 % rn + 1) * osh]
                    src_ap = w8_d.ap()[:, i8 * osh:(i8 + 1) * osh]
                    prune(eng8.dma_start(wt, src_ap))
                    i8 += 1
                if ib * t == xt_split:
                    # absorber for the second xt chunk's DMA wait
                    nc.tensor.ldweights(xt_sb[:, xt_split:xt_split + t])
                for g, (o0, w) in enumerate(groups):
                    nc.tensor.matmul(
                        pys[g][:, :],
                        xt_sb[:, ib * t:(ib + 1) * t],
                        wt[:, o0:o0 + w],
                        start=(ib == 0),
                        stop=(ib == n_ib - 1),
                    )
            # even groups (incl. the last) evict on ACT and ship on the act
            # HWDGE queue — same-engine chains, no cross-engine sem on the
            # critical tail
            y_engs = ([nc.scalar, nc.sync] if wqueue == "hw3"
                      else [nc.sync, nc.sync])
            for g, (o0, w) in enumerate(groups):
                yo = y_sb[:, o0:o0 + w]
                if g % 2 == 0:
                    nc.scalar.activation(
                        yo, pys[g][:], mybir.ActivationFunctionType.Copy)
                else:
                    nc.vector.tensor_copy(yo, pys[g][:])
                y_engs[g % 2].dma_start(y_d.ap()[:, o0:o0 + w], yo)

    nc.compile()
    return nc


def _legalize_waits(nc):
    """TRN2 ISA structs encode a single sync wait. Drop waits implied by
    queue FIFO: SWDGE same-queue DMA writes are ordered by the descriptor
    ring, so a w-load DMA's DMASW lane wait is redundant once its
    cross-engine WAR wait is kept."""
    import bass_rust

    seq_ok = {"InstDrain", "InstEventSemaphore", "InstNoOp", "InstISA",
              "InstCall", "InstUnconditionalBranch", "InstRegisterMove"}
    for fn in nc.m.functions:
        for bb in fn.blocks:
            for ins in bb.instructions:
                nm = type(ins).__name__
                si = ins.sync_info
                if not si or len(si.on_wait) <= 1 or nm in seq_ok:
                    continue
                waits = list(si.on_wait)
                if nm == "InstDMACopy":
                    keep = [w for w in waits
                            if not w.ant_name.startswith("DMASW")]
                    if len(keep) <= 1:
                        ins.sync_info = bass_rust.SyncInfo(
                            on_wait=keep, on_update=list(si.on_update))
                        continue
                raise RuntimeError(
                    f"unlegalizable multi-wait {nm} {ins.name}: "
                    f"{[w.ant_name for w in waits]}")


def _pack_inputs(x, weight, scale, wdt=WDT, osh=OSH, ncores=NCORES, k16=K16):
    """Host-side shard + dequant + repack. Returns per-core input maps.

    Mixed mode: per core, tiles are ranked by fp8e3 quantization error and
    the worst n16 stream as fp16; the program's K-tile order is a host-chosen
    permutation (contraction is order-independent), with xt permuted (and
    renormed) to match.
    """
    import ml_dtypes
    n_ib = NIB
    n_ob = osh // BLOCK
    t = T
    tdts = _tile_dtypes(wdt, n_ib, k16)
    slots16 = [i for i, d in enumerate(tdts) if d == "fp16"]
    slots8 = [i for i, d in enumerate(tdts) if d == "fp8e3"]
    n16 = len(slots16)
    xf = np.asarray(x, dtype=np.float32).reshape(t, I)
    # xt[p, ib*t+tok] = xf[tok, ib*128+p]
    xt_base = np.ascontiguousarray(
        xf.T.reshape(n_ib, BLOCK, t)
    )  # [n_ib, 128, t] fp32 (permute/renorm per core below)
    weight = np.asarray(weight, dtype=np.float32)
    scale = np.asarray(scale, dtype=np.float32)
    in_maps = []
    for c in range(ncores):
        wsh = weight[c * osh:(c + 1) * osh]            # [osh, I]
        ssh = scale[c * n_ob:(c + 1) * n_ob]           # [n_ob, n_ib]
        wd = (wsh.reshape(n_ob, BLOCK, n_ib, BLOCK)
              * ssh[:, None, :, None]).reshape(osh, I)
        # w^T tiles: wt[ib, p, o] = wd[o, ib*128 + p]
        wt = np.ascontiguousarray(wd.T.reshape(n_ib, BLOCK, osh))
        if n16 == n_ib:
            perm16, perm8 = list(range(n_ib)), []
        else:
            # per-(core, K-tile) power-of-two renorm keeps e3m4 blocks out
            # of the subnormal floor; compensated in this core's xt
            amax = np.maximum(np.abs(wt).max(axis=(1, 2)), 1e-30)
            f = np.exp2(np.floor(np.log2(13.0 / amax))).astype(np.float32)
            wq8 = ((wt * f[:, None, None]).astype(ml_dtypes.float8_e3m4)
                   .astype(np.float32) / f[:, None, None])
            err = ((wq8 - wt) ** 2).sum(axis=(1, 2))
            order = np.argsort(-err)
            perm16 = sorted(order[:n16].tolist())
            perm8 = sorted(order[n16:].tolist())
        xt_t = np.empty((n_ib, BLOCK, t), np.float32)
        m = {}
        if n16:
            w16 = np.ascontiguousarray(
                wt[perm16].transpose(1, 0, 2).reshape(BLOCK, n16 * osh)
            ).astype(np.float16)
            for j, ib in enumerate(perm16):
                xt_t[slots16[j]] = xt_base[ib]
            m["w16"] = w16
        if perm8:
            f8 = f[perm8]
            w8 = np.ascontiguousarray(
                (wt[perm8] * f8[:, None, None]).transpose(1, 0, 2)
                .reshape(BLOCK, len(perm8) * osh)
            ).astype(ml_dtypes.float8_e3m4)
            for j, ib in enumerate(perm8):
                xt_t[slots8[j]] = xt_base[ib] / f8[j]
            m["w8"] = w8
        m["xt"] = np.ascontiguousarray(
            xt_t.transpose(1, 0, 2).reshape(BLOCK, n_ib * t)
        ).astype(np.float16)
        in_maps.append(m)
    return in_maps


_NC_CACHE = {}


def _get_nc(**kw):
    key = tuple(sorted(kw.items()))
    if key not in _NC_CACHE:
        _NC_CACHE[key] = build_nc(**kw)
    return _NC_CACHE[key]


def _run(x, weight, scale, trace=False, wdt=WDT, k16=K16, nc_kw=None,
         **trace_kw):
    from concourse.bass_utils import run_bass_kernel_spmd

    nc = _get_nc(wdt=wdt, k16=k16, **(nc_kw or {}))
    in_maps = _pack_inputs(x, weight, scale, wdt=wdt, k16=k16)
    res = run_bass_kernel_spmd(
        nc, in_maps, core_ids=list(range(NCORES)), trace=trace, **trace_kw)
    y = np.concatenate([res.results[c]["y"] for c in range(NCORES)], axis=1)
    return np.ascontiguousarray(y.reshape(B, S, O).astype(np.float32)), res


def kernel(x, weight, scale):
    return _run(x, weight, scale)[0]


# revision 41
# speedup vs baseline: 1.1077x; 1.0159x over previous
"""Trainium2 Bass kernel for nn_Linear_48335561949661.

y = x @ dequant(weight, scale)^T
  x:      [4, 8, 7168] fp32
  weight: [18432, 7168] fp32 (block-dequantized by scale over 128x128 blocks)
  scale:  [144, 56] fp32
  y:      [4, 8, 18432] fp32

Sharding: column-parallel linear — weight/scale sharded along out_features
across 8 cores, x replicated, outputs concatenated on host.

Host packing applies the block-scale dequant and lays the weight shard out
as w^T K-tiles [i-in-block(128 part), o(free)], so the device is a pure
stream: DMA of 56 K-tiles overlapped with 56x5 PSUM-accumulating matmuls
(stationary x^T tile [128, 32], moving w^T strips of <=512 cols — the PSUM
bank cap), then a 5-chunk eviction + output DMA. No on-device transposes
or elementwise dequant (a step-0 scale AP forces the DVE to 1x, ~134us for
the full weight — so the scale multiply stays on host).

Weight stream dtype is mixed per K-tile: most tiles are fp8e3 (e3m4, fed
to the PE directly at 1 col/cycle) with a per-(core, K-tile) power-of-two
renorm absorbed into that core's x^T tile to dodge the e3m4 subnormal
floor; the K16 tiles with the worst fp8 quantization error stream as fp16
(host ranks per core, K-tile order is a host-chosen permutation). rel err
vs the fp32 reference: 1.1e-2 (k16=8), gate 2e-2 — numpy-simulated
bit-exact against hardware.

Queue layout (measured: one SWDGE queue sustains ~390GB/s; HWDGE queues
~145GB/s each, descriptor-dispatch-bound): fp8 pairs + fp16 first-halves
on SWDGE, fp16 second-halves on the act HWDGE queue, xt/y on sync. First
tiles go as halved single-tile DMAs spread over the queues so the PE
starts ~10.7us in; fp8 steady state ships as 2-tile DMAs to halve per-DMA
overhead. PE runs at its floor (~57.5us busy = 129k moving cols at
2.4GHz, LDWEIGHTS hidden); the stream is PE-bound with ~5us of residual
early-fill gaps. ~80us total vs the 250us session-start baseline.
"""

import sys

sys.path.insert(0, "/opt/trn_rl_repo")

import numpy as np

import concourse.bass as bass
import concourse.tile as tile
from concourse import bacc, mybir

FP32 = mybir.dt.float32
FP16 = mybir.dt.float16
FP8E3 = mybir.dt.float8e3
FP8E4 = mybir.dt.float8e4

BLOCK = 128  # dequant block size

# Full-problem constants (hardcoded per contract; kernel.py reads no files)
B, S, I, O = 4, 8, 7168, 18432
NCORES = 8
T = B * S                # 32 tokens
OSH = O // NCORES        # 2304 out rows per core
NIB = I // BLOCK         # 56 K-tiles

# compute dtype for the weight stream: "fp16", "fp8e3", or "mixed"
WDT = "mixed"
K16 = 8         # mixed: K-tiles (of 56) streamed in fp16, rest fp8e3
NDR = 0         # mixed: K-tiles streamed as e4m3 DoubleRow pairs (0.5cyc/col)
WQUEUE = "hw3"  # w queues: "gpsimd" | "hw2" (sync+act) | "hw3" (+gpsimd)


def _tile_dtypes(wdt, n_ib=NIB, k16=K16, ndr=NDR):
    """Per-K-tile dtype list. Mixed spreads the fp16 slots evenly; the host
    ranks tiles by fp8 quantization error and maps the worst into them and
    the ndr lowest-error ones into the trailing DoubleRow slots."""
    if wdt == "fp16":
        return ["fp16"] * n_ib
    if wdt == "fp8e3":
        return ["fp8e3"] * n_ib
    n_nrm = n_ib - ndr
    slots = ["fp8e3"] * n_nrm
    for j in range(k16):
        # spread evenly but keep slot 0 fp8 (smaller first tile -> the
        # first matmul starts sooner)
        slots[int((j + 1) * n_nrm / (k16 + 1))] = "fp16"
    return slots + ["dr"] * ndr


def build_nc(wdt=WDT, osh=OSH, t=T, n_ib=NIB, nw=8, o_split=512, k16=K16,
             ndr=NDR, wqueue=WQUEUE, pair8=True, debug=False):
    """Per-core Bass program (SPMD: same program, 8 data shards)."""
    assert ndr % 2 == 0
    tdts = _tile_dtypes(wdt, n_ib, k16, ndr)
    n16 = sum(1 for d in tdts if d == "fp16")
    n8 = n_ib - n16 - ndr

    nc = bacc.Bacc("TRN2", target_bir_lowering=False, debug=debug)

    w16_d = (nc.dram_tensor("w16", [BLOCK, n16 * osh], FP16,
                            kind="ExternalInput") if n16 else None)
    w8_d = (nc.dram_tensor("w8", [BLOCK, n8 * osh], FP8E3,
                           kind="ExternalInput") if n8 else None)
    w4_d = (nc.dram_tensor("w4", [BLOCK, ndr * osh], FP8E4,
                           kind="ExternalInput") if ndr else None)
    xt4_d = (nc.dram_tensor("xt4", [BLOCK, ndr * t], FP8E4,
                            kind="ExternalInput") if ndr else None)
    # xt packed on host: xt[p, ib*t + tok] = x[tok, ib*128 + p] (renormed,
    # K-tiles permuted to match the w16/w8 slot assignment)
    xt_d = nc.dram_tensor("xt", [BLOCK, n_ib * t], FP16, kind="ExternalInput")
    y_d = nc.dram_tensor("y", [t, osh], FP32, kind="ExternalOutput")

    groups = [(o0, min(o_split, osh - o0)) for o0 in range(0, osh, o_split)]

    with tile.TileContext(nc) as tc:
        with (
            tc.tile_pool(name="const", bufs=1) as const_pool,
            tc.tile_pool(name="psum", bufs=1, space="PSUM") as psum_pool,
        ):
            np8 = max(2, (nw + 1) // 2)  # fp8 ring: pair slots (2 tiles)
            xt_sb = const_pool.tile([BLOCK, n_ib * t], FP16, tag="xt")
            r16 = (const_pool.tile([BLOCK, osh * min(nw, n16)], FP16,
                                   tag="r16", name="r16") if n16 else None)
            r8 = (const_pool.tile(
                [BLOCK, osh * (2 * np8 if pair8 else min(nw, n8))], FP8E3,
                tag="r8", name="r8") if n8 else None)
            np4 = ndr // 2  # all DR pairs resident: DMAs fully prefetch
            r4 = (const_pool.tile([BLOCK, osh * 2 * np4], FP8E4,
                                  tag="r4", name="r4") if ndr else None)
            xt4_sb = (const_pool.tile([BLOCK, ndr * t], FP8E4, tag="xt4",
                                      name="xt4") if ndr else None)
            y_sb = const_pool.tile([t, osh], FP32, tag="ysb")
            if ndr:
                nc.sync.dma_start(xt4_sb[:], xt4_d.ap())
            # xt in two chunks so the first matmuls aren't gated on the
            # full 459KB load; each chunk's PE wait rides a warmup-absorber
            # ldweights so every matmul keeps a single sync wait (ISA limit)
            xt_split = 16 * t
            nc.sync.dma_start(xt_sb[:, 0:xt_split], xt_d.ap()[:, 0:xt_split])
            nc.sync.dma_start(xt_sb[:, xt_split:], xt_d.ap()[:, xt_split:])
            nc.tensor.ldweights(xt_sb[:, 0:t])

            pys = [psum_pool.tile([t, w], FP32, tag=f"py{g}", name=f"py{g}")
                   for g, (_, w) in enumerate(groups)]

            # fp8 bulk on SWDGE (saturates the DMA engine bus ~390GB/s);
            # fp16 tiles alternate across the two HWDGE queues (each caps

            # at ~145GB/s of descriptor dispatch, but their demand is low)
            if wqueue == "gpsimd":
                eng8, eng16s = nc.gpsimd, [nc.gpsimd]
            elif wqueue == "hw2":
                eng8, eng16s = nc.sync, [nc.scalar]
            else:
                eng8, eng16s = nc.gpsimd, [nc.sync, nc.scalar]
            wdma_names = set()

            def prune(dma):
                # drop DMA->DMA WAW vs the slot's previous fill: the reader
                # matmul's WAR edge (kept) already subsumes it
                for dep in list(dma.ins.sync_dependency_names()):
                    if dep in wdma_names:
                        dma.ins.try_remove_dependency(dep)
                wdma_names.add(dma.ins.name)

            half = (osh // 2 // o_split) * o_split
            i16 = i8 = i4 = 0
            for ib, tdt in enumerate(tdts):
                if tdt == "dr":
                    # trailing e4m3 DoubleRow pairs: one DMA + 5 matmuls
                    # cover TWO K-tiles at 0.5 cyc/col. All pairs fit the
                    # ring, so DMAs prefetch during the main stream.
                    pi4, lane4 = divmod(i4, 2)
                    i4 += 1
                    if lane4 == 1:
                        continue  # covered by the pair's lane-0 matmuls
                    if pi4 == 0:
                        # absorber for the xt4 DMA wait (PE reaches this
                        # long after the 32KB load lands)
                        nc.tensor.ldweights(xt4_sb[:, 0:t])
                    base4 = (pi4 % np4) * 2 * osh
                    prune(eng8.dma_start(
                        r4[:, base4:base4 + 2 * osh],
                        w4_d.ap()[:, i4 * osh - osh:(i4 + 1) * osh]))
                    r4_ap = r4[:]
                    xt4_ap = xt4_sb[:]
                    lhsT3 = bass.AP(
                        xt4_ap.tensor, xt4_ap.offset + pi4 * 2 * t,
                        [list(xt4_ap.ap[0]), [t, 2], [1, t]])
                    for g, (o0, w) in enumerate(groups):
                        rhs3 = bass.AP(
                            r4_ap.tensor, r4_ap.offset + base4 + o0,
                            [list(r4_ap.ap[0]), [osh, 2], [1, w]])
                        nc.tensor.matmul(
                            pys[g][:, :], lhsT3, rhs3,
                            start=(ib == 0),
                            stop=(ib + 1 == n_ib - 1),
                            perf_mode=mybir.MatmulPerfMode.DoubleRow,
                        )
                    continue
                if tdt == "fp16":
                    rn = min(nw, n16)
                    wt = r16[:, (i16 % rn) * osh:(i16 % rn + 1) * osh]
                    src_ap = w16_d.ap()[:, i16 * osh:(i16 + 1) * osh]
                    # halves split between SWDGE (not dispatch-limited) and
                    # the act HWDGE queue, at a matmul group boundary so
                    # each matmul keeps a single sync wait
                    if len(eng16s) == 2:
                        prune(eng8.dma_start(
                            wt[:, 0:half], src_ap[:, 0:half]))
                        prune(eng16s[1].dma_start(
                            wt[:, half:], src_ap[:, half:]))
                    else:
                        prune(eng16s[0].dma_start(wt, src_ap))
                    i16 += 1
                elif pair8:
                    # fp8 stream in 2-tile DMAs: halves the per-DMA
                    # overhead (desc-gen, trigger, DGE delay, sem) so the
                    # SWDGE cadence stays ahead of the PE
                    pi, lane = divmod(i8, 2)
                    base = (pi % np8) * 2 * osh
                    wt = r8[:, base + lane * osh:base + (lane + 1) * osh]
                    src_ap = w8_d.ap()[:, i8 * osh:(i8 + 1) * osh]
                    if i8 < 4:
                        # pipeline fill: the first fp8 tiles go as halved
                        # single-tile DMAs, spread over SWDGE and the
                        # second HWDGE queue, so the PE can start early
                        # without outrunning the delivery schedule
                        feng = (eng8 if i8 % 2 == 0 or len(eng16s) < 2
                                else eng16s[1])
                        prune(feng.dma_start(
                            wt[:, 0:half], src_ap[:, 0:half]))
                        prune(feng.dma_start(wt[:, half:], src_ap[:, half:]))
                    elif lane == 0:
                        n_in_pair = min(2, n8 - i8)
                        prune(eng8.dma_start(
                            r8[:, base:base + n_in_pair * osh],
                            w8_d.ap()[:, i8 * osh:(i8 + n_in_pair) * osh]))
                    i8 += 1
                else:
                    rn = min(nw, n8)
                    wt = r8[:, (i8 % rn) * osh:(i8 % rn + 1) * osh]
                    src_ap = w8_d.ap()[:, i8 * osh:(i8 + 1) * osh]
                    prune(eng8.dma_start(wt, src_ap))
                    i8 += 1
                if ib * t == xt_split:
                    # absorber for the second xt chunk's DMA wait
                    nc.tensor.ldweights(xt_sb[:, xt_split:xt_split + t])
                for g, (o0, w) in enumerate(groups):
                    nc.tensor.matmul(
                        pys[g][:, :],
                        xt_sb[:, ib * t:(ib + 1) * t],
                        wt[:, o0:o0 + w],
                        start=(ib == 0),
                        stop=(ib == n_ib - 1),
                    )
            # even groups (incl. the last) evict on ACT and ship on the act
            # HWDGE queue — same-engine chains, no cross-engine sem on the
            # critical tail
            y_engs = ([nc.scalar, nc.sync] if wqueue == "hw3"
                      else [nc.sync, nc.sync])
            for g, (o0, w) in enumerate(groups):
                yo = y_sb[:, o0:o0 + w]
                if g % 2 == 0:
                    nc.scalar.activation(
                        yo, pys[g][:], mybir.ActivationFunctionType.Copy)
                else:
                    nc.vector.tensor_copy(yo, pys[g][:])
                y_engs[g % 2].dma_start(y_d.ap()[:, o0:o0 + w], yo)

    nc.compile()
    return nc


def _legalize_waits(nc):
    """TRN2 ISA structs encode a single sync wait. Drop waits implied by
    queue FIFO: SWDGE same-queue DMA writes are ordered by the descriptor
    ring, so a w-load DMA's DMASW lane wait is redundant once its
    cross-engine WAR wait is kept."""
    import bass_rust

    seq_ok = {"InstDrain", "InstEventSemaphore", "InstNoOp", "InstISA",
              "InstCall", "InstUnconditionalBranch", "InstRegisterMove"}
    for fn in nc.m.functions:
        for bb in fn.blocks:
            for ins in bb.instructions:
                nm = type(ins).__name__
                si = ins.sync_info
                if not si or len(si.on_wait) <= 1 or nm in seq_ok:
                    continue
                waits = list(si.on_wait)
                if nm == "InstDMACopy":
                    keep = [w for w in waits
                            if not w.ant_name.startswith("DMASW")]
                    if len(keep) <= 1:
                        ins.sync_info = bass_rust.SyncInfo(
                            on_wait=keep, on_update=list(si.on_update))
                        continue
                raise RuntimeError(
                    f"unlegalizable multi-wait {nm} {ins.name}: "
                    f"{[w.ant_name for w in waits]}")


def _pack_inputs(x, weight, scale, wdt=WDT, osh=OSH, ncores=NCORES, k16=K16,
                 ndr=NDR):
    """Host-side shard + dequant + repack. Returns per-core input maps.

    Mixed mode: per core, tiles are ranked by fp8e3 quantization error and
    the worst n16 stream as fp16; the program's K-tile order is a host-chosen
    permutation (contraction is order-independent), with xt permuted (and
    renormed) to match.
    """
    import ml_dtypes
    n_ib = NIB
    n_ob = osh // BLOCK
    t = T
    tdts = _tile_dtypes(wdt, n_ib, k16, ndr)
    slots16 = [i for i, d in enumerate(tdts) if d == "fp16"]
    slots8 = [i for i, d in enumerate(tdts) if d == "fp8e3"]
    slots4 = [i for i, d in enumerate(tdts) if d == "dr"]
    n16 = len(slots16)
    xf = np.asarray(x, dtype=np.float32).reshape(t, I)
    # xt[p, ib*t+tok] = xf[tok, ib*128+p]
    xt_base = np.ascontiguousarray(
        xf.T.reshape(n_ib, BLOCK, t)
    )  # [n_ib, 128, t] fp32 (permute/renorm per core below)
    weight = np.asarray(weight, dtype=np.float32)
    scale = np.asarray(scale, dtype=np.float32)
    in_maps = []
    for c in range(ncores):
        wsh = weight[c * osh:(c + 1) * osh]            # [osh, I]
        ssh = scale[c * n_ob:(c + 1) * n_ob]           # [n_ob, n_ib]
        wd = (wsh.reshape(n_ob, BLOCK, n_ib, BLOCK)
              * ssh[:, None, :, None]).reshape(osh, I)
        # w^T tiles: wt[ib, p, o] = wd[o, ib*128 + p]
        wt = np.ascontiguousarray(wd.T.reshape(n_ib, BLOCK, osh))
        if n16 == n_ib:
            perm16, perm8, perm4 = list(range(n_ib)), [], []
        else:
            # per-(core, K-tile) power-of-two renorm keeps e3m4 blocks out
            # of the subnormal floor; compensated in this core's xt
            amax = np.maximum(np.abs(wt).max(axis=(1, 2)), 1e-30)
            f = np.exp2(np.floor(np.log2(13.0 / amax))).astype(np.float32)
            wq8 = ((wt * f[:, None, None]).astype(ml_dtypes.float8_e3m4)
                   .astype(np.float32) / f[:, None, None])
            err = ((wq8 - wt) ** 2).sum(axis=(1, 2))
            order = np.argsort(-err)
            perm16 = sorted(order[:n16].tolist())
            perm4 = sorted(order[n_ib - ndr:].tolist()) if ndr else []
            perm8 = sorted(order[n16:n_ib - len(perm4)].tolist())
        xt_t = np.empty((n_ib, BLOCK, t), np.float32)
        m = {}
        if n16:
            w16 = np.ascontiguousarray(
                wt[perm16].transpose(1, 0, 2).reshape(BLOCK, n16 * osh)
            ).astype(np.float16)
            for j, ib in enumerate(perm16):
                xt_t[slots16[j]] = xt_base[ib]
            m["w16"] = w16
        if perm8:
            f8 = f[perm8]
            w8 = np.ascontiguousarray(
                (wt[perm8] * f8[:, None, None]).transpose(1, 0, 2)
                .reshape(BLOCK, len(perm8) * osh)
            ).astype(ml_dtypes.float8_e3m4)
            for j, ib in enumerate(perm8):
                xt_t[slots8[j]] = xt_base[ib] / f8[j]
            m["w8"] = w8
        if perm4:
            f4 = f[perm4]
            m["w4"] = np.ascontiguousarray(
                (wt[perm4] * f4[:, None, None]).transpose(1, 0, 2)
                .reshape(BLOCK, len(perm4) * osh)
            ).astype(ml_dtypes.float8_e4m3)
            m["xt4"] = np.ascontiguousarray(
                (xt_base[perm4] / f4[:, None, None]).transpose(1, 0, 2)
                .reshape(BLOCK, len(perm4) * t)
            ).astype(ml_dtypes.float8_e4m3)
            for j, ib in enumerate(perm4):
                xt_t[slots4[j]] = 0.0  # DR slots read xt4, not xt
        m["xt"] = np.ascontiguousarray(
            xt_t.transpose(1, 0, 2).reshape(BLOCK, n_ib * t)
        ).astype(np.float16)
        in_maps.append(m)
    return in_maps


_NC_CACHE = {}


def _get_nc(**kw):
    key = tuple(sorted(kw.items()))
    if key not in _NC_CACHE:
        _NC_CACHE[key] = build_nc(**kw)
    return _NC_CACHE[key]


def _run(x, weight, scale, trace=False, wdt=WDT, k16=K16, ndr=NDR,
         nc_kw=None, **trace_kw):
    from concourse.bass_utils import run_bass_kernel_spmd

    nc = _get_nc(wdt=wdt, k16=k16, ndr=ndr, **(nc_kw or {}))
    in_maps = _pack_inputs(x, weight, scale, wdt=wdt, k16=k16, ndr=ndr)
    res = run_bass_kernel_spmd(
        nc, in_maps, core_ids=list(range(NCORES)), trace=trace, **trace_kw)
    y = np.concatenate([res.results[c]["y"] for c in range(NCORES)], axis=1)
    return np.ascontiguousarray(y.reshape(B, S, O).astype(np.float32)), res


def kernel(x, weight, scale):
    return _run(x, weight, scale)[0]
